# revision 20
# baseline (speedup 1.0000x reference)
"""Causal multi-head self-attention (QK-RMSNorm + tanh softcap) on 8 trn2 cores.

Problem (hardcoded): x [2, 2048, 1024], w_q/w_k/w_v/w_o [1024, 1024] fp32,
H=16 heads, dk=64, softcap 50, causal, out = softmax-attn @ w_o.T.

Sharding: batch x head-group hybrid. Core c owns batch c//4 and heads
4*(c%4)..4*(c%4)+3 (256 local dims):
  - w_q/w_k/w_v sliced by rows -> per-core [256, 1024]; host pre-transposes
    and converts to bf16.
  - w_o sliced by columns -> per-core [1024, 256]; host pre-transposes, bf16.
  - x: only the core's batch, host pre-transposed to xT [1024, 2048] bf16.
  - Each core emits a bf16 partial output [2048, 1024] for its batch; host
    sums the 4 head-group partials per batch in fp32.

On-core pipeline (PE matmuls bf16 in / fp32 PSUM accumulate):
  A) per 512-token tile tt: qT/kT = w.T @ x into resident bf16 [d, t] tiles;
     per-head sumsq via tiny N=2 selector matmuls -> packed [128, 32] column
     tile; rsqrt = quake seed + 2 Newton iters on DVE; rstd unpacked to row
     layout via SBUF DMA; RMS applied in place (eyeT broadcast matmul + DVE
     mul). v projected directly in natural [t, d] layout (x chunk as lhsT)
     and written into the augmented PV operand [v_h|1|...] (bf16).
  B) per (head, 512-query slab): scoresT[j, i] = k.T @ q blocks in PSUM,
     tanh softcap -> f16 staging, exp -> bf16 u (scores bounded by the cap:
     no running max), causal via block skip + triangular mask mul on gpsimd;
     PV in [i, dk] orientation: lhsT = u block [j, i], rhs = v_aug [j, 65]
     (moving dim 65 instead of 128 -> half the PE cycles), denominator rides
     as output column 64; normalize via DVE per-partition tensor_scalar while
     copying to bf16; per-128 block PE transpose back to [d, i] for the
     output projection.
  C) out[t, :] = yT.T @ w_oT per 128-token block -> bf16 partial to DRAM.

Emission interleaves phase A of slab tt+1 (and phase C of slab n-1) into
phase B of slab n via a filler queue, so the PE keeps projecting while ACT
(the bottleneck engine: tanh+exp over ~9.2M score elements) never starves.
"""

import sys

for _p in ("/opt/trn_rl_repo",):
    if _p not in sys.path:
        sys.path.insert(0, _p)

from collections import deque

import numpy as np
import ml_dtypes

import concourse.bacc as bacc
import concourse.tile as tile
from concourse import mybir
from concourse.bass_utils import run_bass_kernel_spmd

F32 = mybir.dt.float32
F32R = mybir.dt.float32r
BF16 = mybir.dt.bfloat16
F16 = mybir.dt.float16
AF = mybir.ActivationFunctionType
ALU = mybir.AluOpType

B, S, D = 2, 2048, 1024
H, DK = 16, 64
NCORES = 8
HLOC = 4                    # heads per core
MLOC = HLOC * DK            # 256 local head dims
EPS = 1.1920929e-07
SOFTCAP = 50.0
TANH_SCALE = 1.0 / (8.0 * SOFTCAP)   # 1/sqrt(dk)/softcap inside the tanh

TT = 512                    # token tile (phase A, also query slab)
NTB = S // TT               # 4 slabs
DIAG_OFF = (0, 512, 896, 1152)   # diag j-sub column offsets in u (widths 512/384/256/128)


def build_kernel():
    nc = bacc.Bacc("TRN2", target_bir_lowering=False, debug=False)

    xT = nc.dram_tensor("xT", [D, S], BF16, kind="ExternalInput")
    wqT = nc.dram_tensor("wqT", [D, MLOC], BF16, kind="ExternalInput")
    wkT = nc.dram_tensor("wkT", [D, MLOC], BF16, kind="ExternalInput")
    wvT = nc.dram_tensor("wvT", [D, MLOC], BF16, kind="ExternalInput")
    woT = nc.dram_tensor("woT", [MLOC, D], BF16, kind="ExternalInput")
    out = nc.dram_tensor("out", [S, D], BF16, kind="ExternalOutput")

    xT3 = xT.ap().rearrange("(o p) t -> p o t", p=128)      # [128, 8, 2048]
    out2 = out.ap()

    with tile.TileContext(nc) as tc:
        _emit(nc, tc, xT3, wqT, wkT, wvT, woT, out2)

    nc.compile()
    return nc


def _emit(nc, tc, xT3, wqT, wkT, wvT, woT, out2):
    from contextlib import ExitStack

    ctx = ExitStack()
    with ctx:
        cn = ctx.enter_context(tc.tile_pool(name="cn", bufs=1))
        xload = ctx.enter_context(tc.tile_pool(name="xload", bufs=2))
        wpool = ctx.enter_context(tc.tile_pool(name="wpool", bufs=1))
        res = ctx.enter_context(tc.tile_pool(name="res", bufs=1))
        sqp = ctx.enter_context(tc.tile_pool(name="sqp", bufs=2))
        # packed-rsqrt tiles: partition-split DMA APs confuse tile slot-reuse
        # dep tracking, so give every tag enough bufs that no slot is reused
        nwt = ctx.enter_context(tc.tile_pool(name="nwt", bufs=4))
        up = ctx.enter_context(tc.tile_pool(name="up", bufs=2))
        ysp = ctx.enter_context(tc.tile_pool(name="ysp", bufs=2))
        yttp = ctx.enter_context(tc.tile_pool(name="yttp", bufs=2))
        osp = ctx.enter_context(tc.tile_pool(name="osp", bufs=2))
        rdp = ctx.enter_context(tc.tile_pool(name="rdp", bufs=2))
        # PSUM: sc 2x[128,1024] (4 banks) + yt 2x[128,4,128] (2) + mm 2x (2)
        scp = ctx.enter_context(tc.tile_pool(name="scp", bufs=2, space="PSUM"))
        ytp = ctx.enter_context(tc.tile_pool(name="ytp", bufs=2, space="PSUM"))
        mmp = ctx.enter_context(tc.tile_pool(name="mmp", bufs=2, space="PSUM"))

        # ---- constants ----
        ident_b = cn.tile([128, 128], BF16, tag="ident_b")
        nc.vector.memset(ident_b, 1.0)
        nc.gpsimd.affine_select(
            out=ident_b, in_=ident_b, pattern=[[1, 128]],
            compare_op=ALU.is_equal, fill=0.0, base=0, channel_multiplier=-1,
        )

        # sel[d, r] = 1 iff d//64 == r (within a 128-dim half): head selector
        sel = cn.tile([128, 2], BF16, tag="sel")
        nc.vector.memset(sel, 0.0)
        nc.vector.memset(sel[0:64, 0:1], 1.0)
        nc.vector.memset(sel[64:128, 1:2], 1.0)

        # eyeT[p, f] = 1 iff 0 <= f - 64p < 64 (build via two affine_selects)
        eyeT_f = cn.tile([2, 128], F32, tag="eyeT_f")
        nc.vector.memset(eyeT_f, 1.0)
        nc.gpsimd.affine_select(
            out=eyeT_f, in_=eyeT_f, pattern=[[1, 128]],
            compare_op=ALU.is_ge, fill=0.0, base=0, channel_multiplier=-64,
        )
        nc.gpsimd.affine_select(
            out=eyeT_f, in_=eyeT_f, pattern=[[-1, 128]],
            compare_op=ALU.is_ge, fill=0.0, base=63, channel_multiplier=64,
        )
        eyeT = cn.tile([2, 128], F32R, tag="eyeT")
        nc.vector.tensor_copy(eyeT, eyeT_f)

        # tri[p, f] = 1 if f >= p else 0  (keep i>=j in [j, i] diag tiles)
        tri = cn.tile([128, 128], BF16, tag="tri")
        nc.vector.memset(tri, 1.0)
        nc.gpsimd.affine_select(
            out=tri, in_=tri, pattern=[[1, 128]],
            compare_op=ALU.is_ge, fill=0.0, base=0, channel_multiplier=-1,
        )

        # ---- weights ----
        wq_t = wpool.tile([128, 8, MLOC], BF16, tag="wq")
        nc.sync.dma_start(out=wq_t, in_=wqT.ap().rearrange("(o p) m -> p o m", p=128))
        wk_t = wpool.tile([128, 8, MLOC], BF16, tag="wk")
        nc.sync.dma_start(out=wk_t, in_=wkT.ap().rearrange("(o p) m -> p o m", p=128))
        wv_t = wpool.tile([128, 8, MLOC], BF16, tag="wv")
        nc.sync.dma_start(out=wv_t, in_=wvT.ap().rearrange("(o p) m -> p o m", p=128))
        wo_t = wpool.tile([128, 2, D], BF16, tag="wo")
        nc.sync.dma_start(out=wo_t, in_=woT.ap().rearrange("(c p) od -> p c od", p=128))

        # ---- residents ----
        qTn = [res.tile([128, S], BF16, tag=f"qTn{m}", name=f"qTn{m}") for m in range(2)]
        kTn = [res.tile([128, S], BF16, tag=f"kTn{m}", name=f"kTn{m}") for m in range(2)]
        # vaug: [128 (t within j-block), jb, 4*65] = [v_h0 | 1 | v_h1 | 1 | ...]
        vaug = res.tile([128, S // 128, 4 * 65], BF16, tag="vaug", name="vaug")
        for h in range(HLOC):
            nc.vector.memset(vaug[:, :, 65 * h + 64 : 65 * h + 65], 1.0)

        # ---- phase A pieces ----
        xts = {}
        ssT_sb = {}   # tt -> [128, 2, 2, 4, 2] f32 (hf, w, s, r)
        rows = {}     # (tt, hf, w) -> [2, 4, 128] f32r row tile
        QK_N = {0: qTn, 1: kTn}

        def emit_xt(tt):
            t = xload.tile([128, 8, TT], BF16, tag="xt", name=f"xt{tt}")
            nc.sync.dma_start(out=t, in_=xT3[:, :, tt * TT : (tt + 1) * TT])
            xts[tt] = t

        def emit_proj(tt, w, hf):
            wt = (wq_t, wk_t)[w]
            dest = QK_N[w][hf]
            xt = xts[tt]
            t0 = tt * TT
            ps = mmp.tile([128, TT], F32, tag="mm")
            for kc in range(8):
                nc.tensor.matmul(
                    ps, wt[:, kc, 128 * hf : 128 * hf + 128], xt[:, kc, :],
                    start=(kc == 0), stop=(kc == 7),
                )
            with nc.allow_low_precision(reason="bf16 projection staging"):
                nc.vector.tensor_copy(dest[:, t0 : t0 + TT], ps)
            del ps
            sq = sqp.tile([128, TT], BF16, tag="sq")
            with nc.allow_low_precision(reason="bf16 square for sumsq"):
                nc.vector.tensor_mul(sq, dest[:, t0 : t0 + TT], dest[:, t0 : t0 + TT])
            sp = mmp.tile([128, 4, 2], F32, tag="mm")
            for s in range(4):
                nc.tensor.matmul(
                    sp[:, s, :], sq[:, 128 * s : 128 * s + 128], sel,
                    start=True, stop=True,
                )
            del sq
            if tt not in ssT_sb:
                # free layout (hf, w, r, s): s contiguous so the rstd unpack
                # DMA has a contiguous final dim on both sides
                ssT_sb[tt] = nwt.tile(
                    [128, 2, 2, 2, 4], F32, tag="ssT", bufs=4, name=f"ssT{tt}"
                )
            nc.vector.tensor_copy(
                ssT_sb[tt][:, hf, w, :, :], sp.rearrange("p s r -> p r s")
            )
            del sp

        def emit_newton(tt, hf):
            st = ssT_sb[tt][:, hf].rearrange("p b r s -> p (b r s)")
            v = nwt.tile([128, 16], F32, tag="nv", bufs=8, name=f"nv{tt}_{hf}")
            nc.vector.tensor_scalar(v, st, 1.0 / DK, EPS, ALU.mult, ALU.add)
            y = nwt.tile([128, 16], F32, tag="ny", bufs=8, name=f"ny{tt}_{hf}")
            t1 = nwt.tile([128, 16], F32, tag="nt", bufs=8, name=f"nt{tt}_{hf}")
            nc.vector.tensor_scalar(
                y.bitcast(mybir.dt.int32), v.bitcast(mybir.dt.int32),
                1, None, ALU.logical_shift_right,
            )
            nc.vector.tensor_scalar(
                y.bitcast(mybir.dt.int32), y.bitcast(mybir.dt.int32),
                -1, 0x5F3759DF, ALU.mult, ALU.add,
            )
            # 2 Newton iterations: rel err ~4e-6, far below bf16 rounding
            for _ in range(2):
                nc.vector.tensor_mul(t1, y, y)
                nc.vector.tensor_mul(t1, t1, v)
                nc.vector.tensor_scalar(t1, t1, -0.5, 1.5, ALU.mult, ALU.add)
                nc.vector.tensor_mul(y, y, t1)
            del v, t1
            y4 = y.bitcast(F32R).rearrange("p (b r s) -> p b r s", b=2, r=2, s=4)
            for w in range(2):
                # row tile in (r, p, s) token order: t' = 4*p + s; the
                # rms-mul below reads qTn/kTn through a matching
                # permuted AP, so no reorder pass is needed
                rt = nwt.tile(
                    [2, 128, 4], F32R, tag="row", bufs=16,
                    name=f"row{tt}_{hf}_{w}",
                )
                for r in range(2):
                    nc.sync.dma_start(out=rt[r : r + 1], in_=y4[:, w, r])
                rows[(tt, hf, w)] = rt
            if hf == 1:
                del ssT_sb[tt]

        def emit_rms(tt, w, hf):
            t0 = tt * TT
            rt = rows.pop((tt, hf, w))
            bc = mmp.tile([128, TT], F32, tag="mm")
            nc.tensor.matmul(
                bc, eyeT, rt.rearrange("r p s -> r (p s)"), start=True, stop=True
            )
            # bc's free dim is in t' = 4*p + s order; read the destination
            # through a (p, s)-permuted AP to match
            dest = QK_N[w][hf][:, t0 : t0 + TT].rearrange(
                "d (s p) -> d p s", s=4, p=128
            )
            bc3 = bc.rearrange("d (p s) -> d p s", p=128, s=4)
            with nc.allow_low_precision(reason="bf16 rms apply"):
                nc.vector.tensor_mul(dest, dest, bc3)
            del bc, rt

        def emit_vdirect(tt, hf):
            xt = xts[tt]
            for s in range(4):
                ps = mmp.tile([128, 128], F32, tag="mm")
                for kc in range(8):
                    nc.tensor.matmul(
                        ps, xt[:, kc, 128 * s : 128 * s + 128],
                        wv_t[:, kc, 128 * hf : 128 * hf + 128],
                        start=(kc == 0), stop=(kc == 7),
                    )
                jb = 4 * tt + s
                dst = vaug[:, jb, :].rearrange("p (h c) -> p h c", c=65)[
                    :, 2 * hf : 2 * hf + 2, 0:64
                ]
                src = ps.rearrange("p (h c) -> p h c", c=64)
                with nc.allow_low_precision(reason="bf16 v staging"):
                    nc.vector.tensor_copy(dst, src)
                del ps
            if hf == 1:
                del xts[tt]

        # filler queue: (key, closure). Keys let emit_B force-drain exactly
        # what a head depends on (FIFO order preserves intra-unit deps).
        filler = deque()
        done = set()

        def drain(k):
            for _ in range(min(k, len(filler))):
                key, fn = filler.popleft()
                fn()
                if key is not None:
                    done.add(key)

        def ensure(key):
            while key not in done:
                if not filler:
                    raise RuntimeError(f"filler underrun waiting for {key}")
                drain(1)

        def a_unit(tt):
            # per dim-half: projections -> rsqrt -> v -> rms, so heads 0/1 of
            # the next slab can start before the hf=1 work is even emitted
            L = [(None, lambda: emit_xt(tt))]
            for hf in range(2):
                L.append((None, lambda hf=hf: emit_proj(tt, 0, hf)))
                L.append((None, lambda hf=hf: emit_proj(tt, 1, hf)))
                L.append((None, lambda hf=hf: emit_newton(tt, hf)))
                L.append(((("v", tt, hf)), lambda hf=hf: emit_vdirect(tt, hf)))
                L.append((None, lambda hf=hf: emit_rms(tt, 0, hf)))
                L.append(
                    ((("rms", tt, hf)), lambda hf=hf: emit_rms(tt, 1, hf))
                )
            return L

        # ---- phase B ----
        def qk(sc_slice, hf, hp, jbl, i0, iw):
            nc.tensor.matmul(
                sc_slice,
                kTn[hf][hp : hp + 64, 128 * jbl : 128 * jbl + 128],
                qTn[hf][hp : hp + 64, i0 : i0 + iw],
                start=True, stop=True,
            )

        def emit_B(n, pre_norm_hook=None):
            i0 = n * TT
            ySB = [
                [
                    ysp.tile([128, 128], BF16, tag=f"ysb_{p}_{s}", name=f"ysb_{p}_{s}")
                    for s in range(4)
                ]
                for p in range(2)
            ]
            for h in range(HLOC):
                hf, hp = h // 2, 64 * (h % 2)
                ensure(("rms", n, hf))
                us = []
                for qi in range(n):
                    uraw = up.tile([128, 2048], F16, tag="uraw")
                    for pe_i in range(2):
                        sc = scp.tile([128, 1024], F32, tag="sc")
                        for e2 in range(2):
                            jbl = 4 * qi + 2 * pe_i + e2
                            qk(sc[:, 512 * e2 : 512 * e2 + 512], hf, hp, jbl, i0, TT)
                        nc.scalar.activation(
                            uraw[:, 1024 * pe_i : 1024 * pe_i + 1024], sc,
                            AF.Tanh, scale=TANH_SCALE,
                        )
                        del sc
                    u = up.tile([128, 2048], BF16, tag=f"u{qi}", name=f"u{qi}")
                    nc.scalar.activation(u, uraw, AF.Exp, scale=SOFTCAP)
                    del uraw
                    us.append(u)
                    drain(1)
                # diagonal: 4 j-blocks, staircase widths 512/384/256/128
                uraw = up.tile([128, 1280], F16, tag="uraw")
                sc = scp.tile([128, 1024], F32, tag="sc")
                qk(sc[:, 0:512], hf, hp, 4 * n, i0, 512)
                qk(sc[:, 512:896], hf, hp, 4 * n + 1, i0 + 128, 384)
                nc.scalar.activation(
                    uraw[:, 0:896], sc[:, 0:896], AF.Tanh, scale=TANH_SCALE
                )
                del sc
                sc = scp.tile([128, 1024], F32, tag="sc")
                qk(sc[:, 0:256], hf, hp, 4 * n + 2, i0 + 256, 256)
                qk(sc[:, 256:384], hf, hp, 4 * n + 3, i0 + 384, 128)
                nc.scalar.activation(
                    uraw[:, 896:1280], sc[:, 0:384], AF.Tanh, scale=TANH_SCALE
                )
                del sc
                ud = up.tile([128, 1280], BF16, tag="ud", name="ud")
                nc.scalar.activation(ud, uraw, AF.Exp, scale=SOFTCAP)
                del uraw
                for js in range(4):
                    off = DIAG_OFF[js]
                    nc.gpsimd.tensor_mul(
                        ud[:, off : off + 128], ud[:, off : off + 128], tri
                    )
                drain(1)
                ensure(("v", n, hf))
                # PV sub-major: the PSUM accumulation group of each i-sub
                # must be a run of consecutive matmuls within the yt bank
                # (one open group per bank; other banks may interleave)
                yt = ytp.tile([128, 4, 128], F32, tag="yt")
                for s in range(4):
                    first = True
                    for qi in range(n):
                        for e2 in range(4):
                            nc.tensor.matmul(
                                yt[:, s, 0:65],
                                us[qi][:, 512 * e2 + 128 * s : 512 * e2 + 128 * s + 128],
                                vaug[:, 4 * qi + e2, 65 * h : 65 * h + 65],
                                start=first, stop=False,
                            )
                            first = False
                    for js in range(s + 1):
                        off = DIAG_OFF[js] + 128 * (s - js)
                        nc.tensor.matmul(
                            yt[:, s, 0:65],
                            ud[:, off : off + 128],
                            vaug[:, 4 * n + js, 65 * h : 65 * h + 65],
                            start=first, stop=(js == s),
                        )
                        first = False
                del us, ud
                if h == 0 and pre_norm_hook is not None:
                    pre_norm_hook()
                # normalize head h: rden per query token, fused into the
                # PSUM -> SBUF bf16 copy via per-partition tensor_scalar
                rd = rdp.tile([128, 4, 1], F32, tag="rd")
                with nc.allow_low_precision(reason="softmax denominator recip"):
                    nc.vector.reciprocal(rd, yt[:, :, 64:65])
                for s in range(4):
                    with nc.allow_low_precision(reason="bf16 normalized y"):
                        nc.vector.tensor_scalar(
                            ySB[hf][s][:, hp : hp + 64], yt[:, s, 0:64],
                            rd[:, s, :], None, ALU.mult,
                        )
                del yt, rd
                drain(1)
            return ySB

        def c_unit(n, ySB):
            ytts = {}

            def tr():
                ytt = [
                    yttp.tile([128, TT], BF16, tag=f"ytt{c}", name=f"ytt{c}")
                    for c in range(2)
                ]
                for s in range(4):
                    for p in range(2):
                        tp = mmp.tile([128, 128], BF16, tag="mm")
                        nc.tensor.transpose(tp, ySB[p][s], ident_b)
                        with nc.allow_low_precision(reason="bf16 yT staging"):
                            nc.vector.tensor_copy(
                                ytt[p][:, 128 * s : 128 * s + 128], tp
                            )
                        del tp
                ytts["ytt"] = ytt

            def blk_fn(blk):
                ytt = ytts["ytt"]
                osb = osp.tile([128, D], BF16, tag="os")
                for oh in range(2):
                    cop = mmp.tile([128, 512], F32, tag="mm")
                    nc.tensor.matmul(
                        cop, ytt[0][:, 128 * blk : 128 * blk + 128],
                        wo_t[:, 0, 512 * oh : 512 * oh + 512],
                        start=True, stop=False,
                    )
                    nc.tensor.matmul(
                        cop, ytt[1][:, 128 * blk : 128 * blk + 128],
                        wo_t[:, 1, 512 * oh : 512 * oh + 512],
                        start=False, stop=True,
                    )
                    with nc.allow_low_precision(reason="bf16 partial out"):
                        nc.vector.tensor_copy(osb[:, 512 * oh : 512 * oh + 512], cop)
                    del cop
                r0 = n * TT + 128 * blk
                nc.sync.dma_start(out=out2[r0 : r0 + 128, :], in_=osb)
                del osb

            L = [(None, tr)]
            for blk in range(4):
                L.append(
                    (("c", n) if blk == 3 else None,
                     lambda blk=blk: blk_fn(blk))
                )
            return L

        # ---- main schedule ----
        done.add(("c", -1))
        done.add(("c", -2))
        filler.extend(a_unit(0))
        for n in range(NTB):
            if n + 1 < NTB:
                filler.extend(a_unit(n + 1))
            # phase C of slab n-2 must be fully emitted before this slab's
            # normalize cycles back onto its ySB pool slots (bufs=2)
            ySB = emit_B(n, pre_norm_hook=lambda n=n: ensure(("c", n - 2)))
            filler.extend(c_unit(n, ySB))
        while filler:
            drain(1)


_NC_CACHE = None


def _get_nc():
    global _NC_CACHE
    if _NC_CACHE is None:
        _NC_CACHE = build_kernel()
    return _NC_CACHE


def make_in_maps(x, w_q, w_k, w_v, w_o):
    bf = ml_dtypes.bfloat16
    x = np.asarray(x, dtype=np.float32)
    w_q = np.asarray(w_q, dtype=np.float32)
    w_k = np.asarray(w_k, dtype=np.float32)
    w_v = np.asarray(w_v, dtype=np.float32)
    w_o = np.asarray(w_o, dtype=np.float32)

    xTb = [np.ascontiguousarray(x[b].T.astype(bf)) for b in range(B)]
    in_maps = []
    for c in range(NCORES):
        b, g = divmod(c, NCORES // B)
        hs = slice(MLOC * g, MLOC * (g + 1))
        in_maps.append(
            {
                "xT": xTb[b],
                "wqT": np.ascontiguousarray(w_q[hs, :].T.astype(bf)),
                "wkT": np.ascontiguousarray(w_k[hs, :].T.astype(bf)),
                "wvT": np.ascontiguousarray(w_v[hs, :].T.astype(bf)),
                "woT": np.ascontiguousarray(w_o[:, hs].T.astype(bf)),
            }
        )
    return in_maps


def combine_outputs(results):
    out = np.zeros((B, S, D), dtype=np.float32)
    for c in range(NCORES):
        b = c // (NCORES // B)
        out[b] += np.asarray(results[c]["out"]).astype(np.float32)
    return out


def kernel(x, w_q, w_k, w_v, w_o):
    in_maps = make_in_maps(x, w_q, w_k, w_v, w_o)
    nc = _get_nc()
    res = run_bass_kernel_spmd(nc, in_maps, core_ids=list(range(NCORES)))
    return combine_outputs(res.results)


if __name__ == "__main__":
    rng = np.random.default_rng(0)
    ins = {
        "x": rng.standard_normal((B, S, D), dtype=np.float32),
        "w_q": rng.standard_normal((D, D), dtype=np.float32) * 0.02,
        "w_k": rng.standard_normal((D, D), dtype=np.float32) * 0.02,
        "w_v": rng.standard_normal((D, D), dtype=np.float32) * 0.02,
        "w_o": rng.standard_normal((D, D), dtype=np.float32) * 0.02,
    }
    y = kernel(**ins)
    print("kernel output", y.shape, y.dtype, float(np.abs(y).max()))


# revision 28
# speedup vs baseline: 1.0068x; 1.0068x over previous
"""Causal multi-head self-attention (QK-RMSNorm + tanh softcap) on 8 trn2 cores.

Problem (hardcoded): x [2, 2048, 1024], w_q/w_k/w_v/w_o [1024, 1024] fp32,
H=16 heads, dk=64, softcap 50, causal, out = softmax-attn @ w_o.T.

Sharding: batch x head-group hybrid. Core c owns batch c//4 and heads
4*(c%4)..4*(c%4)+3 (256 local dims):
  - w_q/w_k/w_v sliced by rows -> per-core [256, 1024]; host pre-transposes
    and converts to bf16.
  - w_o sliced by columns -> per-core [1024, 256]; host pre-transposes, bf16.
  - x: only the core's batch, host pre-transposed to xT [1024, 2048] bf16.
  - Each core emits a bf16 partial output [2048, 1024] for its batch; host
    sums the 4 head-group partials per batch in fp32.

On-core pipeline (PE matmuls bf16 in / fp32 PSUM accumulate):
  A) per 512-token tile tt: qT/kT = w.T @ x into resident bf16 [d, t] tiles;
     per-head sumsq via tiny N=2 selector matmuls -> packed [128, 32] column
     tile; rsqrt = quake seed + 2 Newton iters on DVE; rstd unpacked to row
     layout via SBUF DMA; RMS applied in place (eyeT broadcast matmul + DVE
     mul). v projected directly in natural [t, d] layout (x chunk as lhsT)
     and written into the augmented PV operand [v_h|1|...] (bf16).
  B) per (head, 512-query slab): scoresT[j, i] = k.T @ q blocks in PSUM,
     tanh softcap -> f16 staging, exp -> bf16 u (scores bounded by the cap:
     no running max), causal via block skip + triangular mask mul on gpsimd;
     PV in [i, dk] orientation: lhsT = u block [j, i], rhs = v_aug [j, 65]
     (moving dim 65 instead of 128 -> half the PE cycles), denominator rides
     as output column 64; normalize via DVE per-partition tensor_scalar while
     copying to bf16; per-128 block PE transpose back to [d, i] for the
     output projection.
  C) out[t, :] = yT.T @ w_oT per 128-token block -> bf16 partial to DRAM.

Emission interleaves phase A of slab tt+1 (and phase C of slab n-1) into
phase B of slab n via a filler queue, so the PE keeps projecting while ACT
(the bottleneck engine: tanh+exp over ~9.2M score elements) never starves.
"""

import sys

for _p in ("/opt/trn_rl_repo",):
    if _p not in sys.path:
        sys.path.insert(0, _p)

from collections import deque

import numpy as np
import ml_dtypes

import concourse.bacc as bacc
import concourse.tile as tile
from concourse import mybir
from concourse.bass_utils import run_bass_kernel_spmd

F32 = mybir.dt.float32
F32R = mybir.dt.float32r
BF16 = mybir.dt.bfloat16
F16 = mybir.dt.float16
AF = mybir.ActivationFunctionType
ALU = mybir.AluOpType

B, S, D = 2, 2048, 1024
H, DK = 16, 64
NCORES = 8
HLOC = 4                    # heads per core
MLOC = HLOC * DK            # 256 local head dims
EPS = 1.1920929e-07
SOFTCAP = 50.0
TANH_SCALE = 1.0 / (8.0 * SOFTCAP)   # 1/sqrt(dk)/softcap inside the tanh

TT = 512                    # token tile (phase A, also query slab)
NTB = S // TT               # 4 slabs
# diag j-sub column offsets in u; j-sub 3 (128 wide) sits at 896 and j-sub 2
# (256 wide) at 1024 so every QK matmul stays within one psum bank
DIAG_OFF = (0, 512, 1024, 896)


def build_kernel():
    nc = bacc.Bacc("TRN2", target_bir_lowering=False, debug=False)

    xT = nc.dram_tensor("xT", [D, S], BF16, kind="ExternalInput")
    wqT = nc.dram_tensor("wqT", [D, MLOC], BF16, kind="ExternalInput")
    wkT = nc.dram_tensor("wkT", [D, MLOC], BF16, kind="ExternalInput")
    wvT = nc.dram_tensor("wvT", [D, MLOC], BF16, kind="ExternalInput")
    woT = nc.dram_tensor("woT", [MLOC, D], BF16, kind="ExternalInput")
    out = nc.dram_tensor("out", [S, D], BF16, kind="ExternalOutput")

    xT3 = xT.ap().rearrange("(o p) t -> p o t", p=128)      # [128, 8, 2048]
    out2 = out.ap()

    with tile.TileContext(nc) as tc:
        _emit(nc, tc, xT3, wqT, wkT, wvT, woT, out2)

    nc.compile()
    return nc


def _emit(nc, tc, xT3, wqT, wkT, wvT, woT, out2):
    from contextlib import ExitStack

    ctx = ExitStack()
    with ctx:
        cn = ctx.enter_context(tc.tile_pool(name="cn", bufs=1))
        xload = ctx.enter_context(tc.tile_pool(name="xload", bufs=2))
        wpool = ctx.enter_context(tc.tile_pool(name="wpool", bufs=1))
        res = ctx.enter_context(tc.tile_pool(name="res", bufs=1))
        sqp = ctx.enter_context(tc.tile_pool(name="sqp", bufs=2))
        # packed-rsqrt tiles: partition-split DMA APs confuse tile slot-reuse
        # dep tracking, so give every tag enough bufs that no slot is reused
        nwt = ctx.enter_context(tc.tile_pool(name="nwt", bufs=4))
        up = ctx.enter_context(tc.tile_pool(name="up", bufs=2))
        ysp = ctx.enter_context(tc.tile_pool(name="ysp", bufs=2))
        yttp = ctx.enter_context(tc.tile_pool(name="yttp", bufs=2))
        osp = ctx.enter_context(tc.tile_pool(name="osp", bufs=2))
        rdp = ctx.enter_context(tc.tile_pool(name="rdp", bufs=2))
        # PSUM: sc 1x[128,2048] (4 banks) + yt 2x[128,4,128] (2) + mm 2x (2)
        scp = ctx.enter_context(tc.tile_pool(name="scp", bufs=1, space="PSUM"))
        ytp = ctx.enter_context(tc.tile_pool(name="ytp", bufs=2, space="PSUM"))
        mmp = ctx.enter_context(tc.tile_pool(name="mmp", bufs=2, space="PSUM"))

        # ---- constants ----
        ident_b = cn.tile([128, 128], BF16, tag="ident_b")
        nc.vector.memset(ident_b, 1.0)
        nc.gpsimd.affine_select(
            out=ident_b, in_=ident_b, pattern=[[1, 128]],
            compare_op=ALU.is_equal, fill=0.0, base=0, channel_multiplier=-1,
        )

        # sel[d, r] = 1 iff d//64 == r (within a 128-dim half): head selector
        sel = cn.tile([128, 2], BF16, tag="sel")
        nc.vector.memset(sel, 0.0)
        nc.vector.memset(sel[0:64, 0:1], 1.0)
        nc.vector.memset(sel[64:128, 1:2], 1.0)

        # eyeT[p, f] = 1 iff 0 <= f - 64p < 64 (build via two affine_selects)
        eyeT_f = cn.tile([2, 128], F32, tag="eyeT_f")
        nc.vector.memset(eyeT_f, 1.0)
        nc.gpsimd.affine_select(
            out=eyeT_f, in_=eyeT_f, pattern=[[1, 128]],
            compare_op=ALU.is_ge, fill=0.0, base=0, channel_multiplier=-64,
        )
        nc.gpsimd.affine_select(
            out=eyeT_f, in_=eyeT_f, pattern=[[-1, 128]],
            compare_op=ALU.is_ge, fill=0.0, base=63, channel_multiplier=64,
        )
        eyeT = cn.tile([2, 128], F32R, tag="eyeT")
        nc.vector.tensor_copy(eyeT, eyeT_f)

        # tri[p, f] = 1 if f >= p else 0  (keep i>=j in [j, i] diag tiles)
        tri = cn.tile([128, 128], BF16, tag="tri")
        nc.vector.memset(tri, 1.0)
        nc.gpsimd.affine_select(
            out=tri, in_=tri, pattern=[[1, 128]],
            compare_op=ALU.is_ge, fill=0.0, base=0, channel_multiplier=-1,
        )

        # ---- weights ----
        wq_t = wpool.tile([128, 8, MLOC], BF16, tag="wq")
        nc.sync.dma_start(out=wq_t, in_=wqT.ap().rearrange("(o p) m -> p o m", p=128))
        wk_t = wpool.tile([128, 8, MLOC], BF16, tag="wk")
        nc.sync.dma_start(out=wk_t, in_=wkT.ap().rearrange("(o p) m -> p o m", p=128))
        wv_t = wpool.tile([128, 8, MLOC], BF16, tag="wv")
        nc.sync.dma_start(out=wv_t, in_=wvT.ap().rearrange("(o p) m -> p o m", p=128))
        wo_t = wpool.tile([128, 2, D], BF16, tag="wo")
        nc.sync.dma_start(out=wo_t, in_=woT.ap().rearrange("(c p) od -> p c od", p=128))

        # ---- residents ----
        qTn = [res.tile([128, S], BF16, tag=f"qTn{m}", name=f"qTn{m}") for m in range(2)]
        kTn = [res.tile([128, S], BF16, tag=f"kTn{m}", name=f"kTn{m}") for m in range(2)]
        # vaug: [128 (t within j-block), jb, 4*65] = [v_h0 | 1 | v_h1 | 1 | ...]
        vaug = res.tile([128, S // 128, 4 * 65], BF16, tag="vaug", name="vaug")
        for h in range(HLOC):
            nc.vector.memset(vaug[:, :, 65 * h + 64 : 65 * h + 65], 1.0)

        # ---- phase A pieces ----
        xts = {}
        ssT_sb = {}   # tt -> [128, 2, 2, 4, 2] f32 (hf, w, s, r)
        rows = {}     # (tt, hf, w) -> [2, 4, 128] f32r row tile
        QK_N = {0: qTn, 1: kTn}

        def emit_xt(tt):
            # 8 per-chunk DMAs run in parallel across queues, so the first
            # projection can start ~2us after issue instead of ~4.5us
            t = xload.tile([128, 8, TT], BF16, tag="xt", name=f"xt{tt}")
            for kc in range(8):
                nc.sync.dma_start(
                    out=t[:, kc, :], in_=xT3[:, kc, tt * TT : (tt + 1) * TT]
                )
            xts[tt] = t

        def emit_proj(tt, w, hf):
            wt = (wq_t, wk_t)[w]
            dest = QK_N[w][hf]
            xt = xts[tt]
            t0 = tt * TT
            ps = mmp.tile([128, TT], F32, tag="mm")
            for kc in range(8):
                nc.tensor.matmul(
                    ps, wt[:, kc, 128 * hf : 128 * hf + 128], xt[:, kc, :],
                    start=(kc == 0), stop=(kc == 7),
                )
            with nc.allow_low_precision(reason="bf16 projection staging"):
                nc.vector.tensor_copy(dest[:, t0 : t0 + TT], ps)
            del ps
            sq = sqp.tile([128, TT], BF16, tag="sq")
            with nc.allow_low_precision(reason="bf16 square for sumsq"):
                nc.vector.tensor_mul(sq, dest[:, t0 : t0 + TT], dest[:, t0 : t0 + TT])
            sp = mmp.tile([128, 4, 2], F32, tag="mm")
            for s in range(4):
                nc.tensor.matmul(
                    sp[:, s, :], sq[:, 128 * s : 128 * s + 128], sel,
                    start=True, stop=True,
                )
            del sq
            if tt not in ssT_sb:
                # free layout (hf, w, r, s): s contiguous so the rstd unpack
                # DMA has a contiguous final dim on both sides
                ssT_sb[tt] = nwt.tile(
                    [128, 2, 2, 2, 4], F32, tag="ssT", bufs=4, name=f"ssT{tt}"
                )
            nc.vector.tensor_copy(
                ssT_sb[tt][:, hf, w, :, :], sp.rearrange("p s r -> p r s")
            )
            del sp

        def emit_newton(tt, hf):
            st = ssT_sb[tt][:, hf].rearrange("p b r s -> p (b r s)")
            v = nwt.tile([128, 16], F32, tag="nv", bufs=8, name=f"nv{tt}_{hf}")
            nc.vector.tensor_scalar(v, st, 1.0 / DK, EPS, ALU.mult, ALU.add)
            y = nwt.tile([128, 16], F32, tag="ny", bufs=8, name=f"ny{tt}_{hf}")
            t1 = nwt.tile([128, 16], F32, tag="nt", bufs=8, name=f"nt{tt}_{hf}")
            nc.vector.tensor_scalar(
                y.bitcast(mybir.dt.int32), v.bitcast(mybir.dt.int32),
                1, None, ALU.logical_shift_right,
            )
            nc.vector.tensor_scalar(
                y.bitcast(mybir.dt.int32), y.bitcast(mybir.dt.int32),
                -1, 0x5F3759DF, ALU.mult, ALU.add,
            )
            # 2 Newton iterations: rel err ~4e-6, far below bf16 rounding
            for _ in range(2):
                nc.vector.tensor_mul(t1, y, y)
                nc.vector.tensor_mul(t1, t1, v)
                nc.vector.tensor_scalar(t1, t1, -0.5, 1.5, ALU.mult, ALU.add)
                nc.vector.tensor_mul(y, y, t1)
            del v, t1
            y4 = y.bitcast(F32R).rearrange("p (b r s) -> p b r s", b=2, r=2, s=4)
            for w in range(2):
                # row tile in (r, p, s) token order: t' = 4*p + s; the
                # rms-mul below reads qTn/kTn through a matching
                # permuted AP, so no reorder pass is needed
                rt = nwt.tile(
                    [2, 128, 4], F32R, tag="row", bufs=16,
                    name=f"row{tt}_{hf}_{w}",
                )
                for r in range(2):
                    nc.sync.dma_start(out=rt[r : r + 1], in_=y4[:, w, r])
                rows[(tt, hf, w)] = rt
            if hf == 1:
                del ssT_sb[tt]

        def emit_rms(tt, w, hf):
            t0 = tt * TT
            rt = rows.pop((tt, hf, w))
            bc = mmp.tile([128, TT], F32, tag="mm")
            nc.tensor.matmul(
                bc, eyeT, rt.rearrange("r p s -> r (p s)"), start=True, stop=True
            )
            # bc's free dim is in t' = 4*p + s order; read the destination
            # through a (p, s)-permuted AP to match
            dest = QK_N[w][hf][:, t0 : t0 + TT].rearrange(
                "d (s p) -> d p s", s=4, p=128
            )
            bc3 = bc.rearrange("d (p s) -> d p s", p=128, s=4)
            with nc.allow_low_precision(reason="bf16 rms apply"):
                nc.vector.tensor_mul(dest, dest, bc3)
            del bc, rt

        def emit_vdirect(tt, hf):
            xt = xts[tt]
            for s in range(4):
                ps = mmp.tile([128, 128], F32, tag="mm")
                for kc in range(8):
                    nc.tensor.matmul(
                        ps, xt[:, kc, 128 * s : 128 * s + 128],
                        wv_t[:, kc, 128 * hf : 128 * hf + 128],
                        start=(kc == 0), stop=(kc == 7),
                    )
                jb = 4 * tt + s
                dst = vaug[:, jb, :].rearrange("p (h c) -> p h c", c=65)[
                    :, 2 * hf : 2 * hf + 2, 0:64
                ]
                src = ps.rearrange("p (h c) -> p h c", c=64)
                with nc.allow_low_precision(reason="bf16 v staging"):
                    nc.vector.tensor_copy(dst, src)
                del ps
            if hf == 1:
                del xts[tt]

        # filler queue: (key, closure). Keys let emit_B force-drain exactly
        # what a head depends on (FIFO order preserves intra-unit deps).
        filler = deque()
        done = set()

        def drain(k):
            for _ in range(min(k, len(filler))):
                key, fn = filler.popleft()
                fn()
                if key is not None:
                    done.add(key)

        def ensure(key):
            while key not in done:
                if not filler:
                    raise RuntimeError(f"filler underrun waiting for {key}")
                drain(1)

        def a_unit(tt):
            # per dim-half: projections -> rsqrt -> v -> rms, so heads 0/1 of
            # the next slab can start before the hf=1 work is even emitted
            L = []
            for hf in range(2):
                L.append((None, lambda hf=hf: emit_proj(tt, 0, hf)))
                L.append((None, lambda hf=hf: emit_proj(tt, 1, hf)))
                L.append((None, lambda hf=hf: emit_newton(tt, hf)))
                L.append(((("v", tt, hf)), lambda hf=hf: emit_vdirect(tt, hf)))
                L.append((None, lambda hf=hf: emit_rms(tt, 0, hf)))
                L.append(
                    ((("rms", tt, hf)), lambda hf=hf: emit_rms(tt, 1, hf))
                )
            return L

        # ---- phase B ----
        def qk(sc_slice, hf, hp, jbl, i0, iw):
            nc.tensor.matmul(
                sc_slice,
                kTn[hf][hp : hp + 64, 128 * jbl : 128 * jbl + 128],
                qTn[hf][hp : hp + 64, i0 : i0 + iw],
                start=True, stop=True,
            )

        def emit_B(n, pre_norm_hook=None):
            i0 = n * TT
            ySB = [
                [
                    ysp.tile([128, 128], BF16, tag=f"ysb_{p}_{s}", name=f"ysb_{p}_{s}")
                    for s in range(4)
                ]
                for p in range(2)
            ]
            for h in range(HLOC):
                hf, hp = h // 2, 64 * (h % 2)
                ensure(("rms", n, hf))
                us = []
                for qi in range(n):
                    uraw = up.tile([128, 2048], F16, tag="uraw")
                    sc = scp.tile([128, 2048], F32, tag="sc")
                    for e2 in range(4):
                        jbl = 4 * qi + e2
                        qk(sc[:, 512 * e2 : 512 * e2 + 512], hf, hp, jbl, i0, TT)
                    nc.scalar.activation(uraw, sc, AF.Tanh, scale=TANH_SCALE)
                    del sc
                    u = up.tile([128, 2048], BF16, tag=f"u{qi}", name=f"u{qi}")
                    nc.scalar.activation(u, uraw, AF.Exp, scale=SOFTCAP)
                    del uraw
                    us.append(u)
                    drain(1)
                # diagonal: 4 j-blocks, staircase widths 512/384/256/128
                uraw = up.tile([128, 1280], F16, tag="uraw")
                sc = scp.tile([128, 2048], F32, tag="sc")
                qk(sc[:, 0:512], hf, hp, 4 * n, i0, 512)
                qk(sc[:, 512:896], hf, hp, 4 * n + 1, i0 + 128, 384)
                qk(sc[:, 896:1024], hf, hp, 4 * n + 3, i0 + 384, 128)
                qk(sc[:, 1024:1280], hf, hp, 4 * n + 2, i0 + 256, 256)
                nc.scalar.activation(
                    uraw, sc[:, 0:1280], AF.Tanh, scale=TANH_SCALE
                )
                del sc
                ud = up.tile([128, 1280], BF16, tag="ud", name="ud")
                nc.scalar.activation(ud, uraw, AF.Exp, scale=SOFTCAP)
                del uraw
                for js in range(4):
                    off = DIAG_OFF[js]
                    nc.gpsimd.tensor_mul(
                        ud[:, off : off + 128], ud[:, off : off + 128], tri
                    )
                drain(1)
                ensure(("v", n, hf))
                # PV sub-major: the PSUM accumulation group of each i-sub
                # must be a run of consecutive matmuls within the yt bank
                # (one open group per bank; other banks may interleave)
                yt = ytp.tile([128, 4, 128], F32, tag="yt")
                for s in range(4):
                    first = True
                    for qi in range(n):
                        for e2 in range(4):
                            nc.tensor.matmul(
                                yt[:, s, 0:65],
                                us[qi][:, 512 * e2 + 128 * s : 512 * e2 + 128 * s + 128],
                                vaug[:, 4 * qi + e2, 65 * h : 65 * h + 65],
                                start=first, stop=False,
                            )
                            first = False
                    for js in range(s + 1):
                        off = DIAG_OFF[js] + 128 * (s - js)
                        nc.tensor.matmul(
                            yt[:, s, 0:65],
                            ud[:, off : off + 128],
                            vaug[:, 4 * n + js, 65 * h : 65 * h + 65],
                            start=first, stop=(js == s),
                        )
                        first = False
                del us, ud
                if h == 0 and pre_norm_hook is not None:
                    pre_norm_hook()
                # normalize head h: rden per query token, fused into the
                # PSUM -> SBUF bf16 copy via per-partition tensor_scalar
                rd = rdp.tile([128, 4, 1], F32, tag="rd")
                with nc.allow_low_precision(reason="softmax denominator recip"):
                    nc.vector.reciprocal(rd, yt[:, :, 64:65])
                for s in range(4):
                    with nc.allow_low_precision(reason="bf16 normalized y"):
                        nc.vector.tensor_scalar(
                            ySB[hf][s][:, hp : hp + 64], yt[:, s, 0:64],
                            rd[:, s, :], None, ALU.mult,
                        )
                del yt, rd
                drain(1)
            return ySB

        def c_unit(n, ySB):
            ytts = {}

            def tr():
                ytt = [
                    yttp.tile([128, TT], BF16, tag=f"ytt{c}", name=f"ytt{c}")
                    for c in range(2)
                ]
                for s in range(4):
                    for p in range(2):
                        tp = mmp.tile([128, 128], BF16, tag="mm")
                        nc.tensor.transpose(tp, ySB[p][s], ident_b)
                        with nc.allow_low_precision(reason="bf16 yT staging"):
                            nc.vector.tensor_copy(
                                ytt[p][:, 128 * s : 128 * s + 128], tp
                            )
                        del tp
                ytts["ytt"] = ytt

            def blk_fn(blk):
                ytt = ytts["ytt"]
                osb = osp.tile([128, D], BF16, tag="os")
                for oh in range(2):
                    cop = mmp.tile([128, 512], F32, tag="mm")
                    nc.tensor.matmul(
                        cop, ytt[0][:, 128 * blk : 128 * blk + 128],
                        wo_t[:, 0, 512 * oh : 512 * oh + 512],
                        start=True, stop=False,
                    )
                    nc.tensor.matmul(
                        cop, ytt[1][:, 128 * blk : 128 * blk + 128],
                        wo_t[:, 1, 512 * oh : 512 * oh + 512],
                        start=False, stop=True,
                    )
                    with nc.allow_low_precision(reason="bf16 partial out"):
                        nc.vector.tensor_copy(osb[:, 512 * oh : 512 * oh + 512], cop)
                    del cop
                r0 = n * TT + 128 * blk
                nc.sync.dma_start(out=out2[r0 : r0 + 128, :], in_=osb)
                del osb

            L = [(None, tr)]
            for blk in range(4):
                L.append(
                    (("c", n) if blk == 3 else None,
                     lambda blk=blk: blk_fn(blk))
                )
            return L

        # ---- main schedule ----
        done.add(("c", -1))
        done.add(("c", -2))
        emit_xt(0)
        filler.extend(a_unit(0))
        for n in range(NTB):
            if n + 1 < NTB:
                emit_xt(n + 1)
                filler.extend(a_unit(n + 1))
            # phase C of slab n-2 must be fully emitted before this slab's
            # normalize cycles back onto its ySB pool slots (bufs=2)
            ySB = emit_B(n, pre_norm_hook=lambda n=n: ensure(("c", n - 2)))
            filler.extend(c_unit(n, ySB))
        while filler:
            drain(1)


_NC_CACHE = None


def _get_nc():
    global _NC_CACHE
    if _NC_CACHE is None:
        _NC_CACHE = build_kernel()
    return _NC_CACHE


def make_in_maps(x, w_q, w_k, w_v, w_o):
    bf = ml_dtypes.bfloat16
    x = np.asarray(x, dtype=np.float32)
    w_q = np.asarray(w_q, dtype=np.float32)
    w_k = np.asarray(w_k, dtype=np.float32)
    w_v = np.asarray(w_v, dtype=np.float32)
    w_o = np.asarray(w_o, dtype=np.float32)

    xTb = [np.ascontiguousarray(x[b].T.astype(bf)) for b in range(B)]
    in_maps = []
    for c in range(NCORES):
        b, g = divmod(c, NCORES // B)
        hs = slice(MLOC * g, MLOC * (g + 1))
        in_maps.append(
            {
                "xT": xTb[b],
                "wqT": np.ascontiguousarray(w_q[hs, :].T.astype(bf)),
                "wkT": np.ascontiguousarray(w_k[hs, :].T.astype(bf)),
                "wvT": np.ascontiguousarray(w_v[hs, :].T.astype(bf)),
                "woT": np.ascontiguousarray(w_o[:, hs].T.astype(bf)),
            }
        )
    return in_maps


def combine_outputs(results):
    out = np.zeros((B, S, D), dtype=np.float32)
    for c in range(NCORES):
        b = c // (NCORES // B)
        out[b] += np.asarray(results[c]["out"]).astype(np.float32)
    return out


def kernel(x, w_q, w_k, w_v, w_o):
    in_maps = make_in_maps(x, w_q, w_k, w_v, w_o)
    nc = _get_nc()
    res = run_bass_kernel_spmd(nc, in_maps, core_ids=list(range(NCORES)))
    return combine_outputs(res.results)


if __name__ == "__main__":
    rng = np.random.default_rng(0)
    ins = {
        "x": rng.standard_normal((B, S, D), dtype=np.float32),
        "w_q": rng.standard_normal((D, D), dtype=np.float32) * 0.02,
        "w_k": rng.standard_normal((D, D), dtype=np.float32) * 0.02,
        "w_v": rng.standard_normal((D, D), dtype=np.float32) * 0.02,
        "w_o": rng.standard_normal((D, D), dtype=np.float32) * 0.02,
    }
    y = kernel(**ins)
    print("kernel output", y.shape, y.dtype, float(np.abs(y).max()))
